# revision 14
# baseline (speedup 1.0000x reference)
"""LatticeLSTM (BiLSTM w/ word cells) Trainium2 kernel.

Sharding: time-sharded across 8 cores with balanced windows. Core k
computes local window [63k, 63k+71) of the 512-step scan for ALL 64
lanes (32 batch fw + 32 batch bw). Core 0 owns all 71 of its steps;
cores 1-7 warm up from zero state for W=8 steps and own the last 63.
(Truncation error ~4e-3 rel; coupled forget gate contracts ~0.5/step.)

Key latency optimization (the scan is dependency-bound, no engine is
>40% busy): the word-cell (V2) chain is taken OFF the per-step critical
path via the pc-lag identity: a merge at step t implies no word started
at t-1 (a word starting at t-1 sets pcnt>=1, killing a merge at t), so
every consumer of pc at step t (alpha pre-gate and the merge select)
may read pc(t-2) instead of pc(t-1). The critical cycle is then only
V1(t) -> whh matmuls -> V1(t+1); the word-cell gates, pc update, and
the merge operand q(t+1)=select(m,pc,c) all compute in engine idle time
with ~1.5 steps of slack. pc/c/q are ping-pong buffers (parity t%2).

Device layout: "layout B" -- gate index on SBUF partitions, lanes on
the free dim; weight-stationary matmuls out[gates,lanes] = W^T @ h.
pg PSUM bank chunk order is [i(2) alpha(2) o(2) g(2)] so ONE Sigmoid
covers i+alpha+o and the s=Sig(sig_i - sig_alpha) chain starts early.
Blends use tensor_copy + copy_predicated (masks are 0/1 f32).

Per-iteration emission (software-pipelined): pg(t+1) prep matmuls
[bias-selector, x-side, whh (waits h(t)), alpha (stop)], V1(t) EW chain,
pw(t) matmuls, V2(t-1) EW + q(t+1) blend. Scalar queue order
[Sig(i,al,o), Tanh(g), Sig(s), Sig(iw,fw)(t-1), Tanh(gw)(t-1), Tanh(c)]
keeps every act within its dependency slack.
"""

import numpy as np
import ml_dtypes

import concourse.bass as bass
import concourse.bacc as bacc
import concourse.tile as tile
from concourse import mybir
from concourse.bass_utils import run_bass_kernel_spmd

B, S, E, H, V, L = 32, 512, 128, 256, 21128, 32
NCORES = 8
WARM = 8
T = 64 + 7 * WARM // 8      # 71 local steps per core
OWNK = T - WARM             # 63 owned steps on cores 1-7
LANES = 64                  # 32 fw + 32 bw
NIDX = T * LANES            # gathered rows per table
NT = 12                     # tag matmul steps per chunk

f32 = mybir.dt.float32
bf16 = mybir.dt.bfloat16
i8 = mybir.dt.int8
Sig = mybir.ActivationFunctionType.Sigmoid
Tanh = mybir.ActivationFunctionType.Tanh

bf = ml_dtypes.bfloat16

_CACHE = {}



def _chunk_bcast(ap2, nchunk=2):
    """[128, 64] AP -> [128, nchunk, 64] with zero-stride chunk dim."""
    return bass.AP(tensor=ap2.tensor, offset=ap2.offset,
                   ap=[ap2.ap[0], [0, nchunk], ap2.ap[1]])


def _build_bass():
    nc = bacc.Bacc(None, target_bir_lowering=False)

    def inp(name, shape, dtype):
        return nc.declare_dram_parameter(name, list(shape), dtype, isOutput=False)

    xT_d = inp("x_T", [128, NIDX], bf16)
    weT_d = inp("we_T", [128, NIDX], bf16)
    wih_d = {d: inp(f"wih_{d}", [E, 6, 128], bf16) for d in "fb"}
    awih_d = {d: inp(f"awih_{d}", [E, 2, 128], bf16) for d in "fb"}
    wwih_d = {d: inp(f"wwih_{d}", [E, 6, 128], bf16) for d in "fb"}
    whh_d = {d: inp(f"whh_{d}", [128, 2, 6, 128], bf16) for d in "fb"}
    wwhh_d = {d: inp(f"wwhh_{d}", [128, 2, 6, 128], bf16) for d in "fb"}
    awhh_d = {d: inp(f"awhh_{d}", [128, 2, 2, 128], bf16) for d in "fb"}
    bg_d = {d: inp(f"biasg_{d}", [6, 128], bf16) for d in "fb"}
    ba_d = {d: inp(f"biasa_{d}", [2, 128], bf16) for d in "fb"}
    bw_d = {d: inp(f"biasw_{d}", [6, 128], bf16) for d in "fb"}
    sel2_d = inp("sel2", [2, 2 * 32], bf16)
    selw_d = inp("selw", [6, 6 * 32], bf16)
    maskm_d = inp("mask_m", [T, LANES], i8)
    maskw_d = inp("mask_w", [T, LANES], i8)
    tagw_d = inp("tagw", [128, 2, 2, 32], bf16)

    out_d = nc.declare_dram_parameter("out_tags", [2, 32, T * 32], f32, isOutput=True)

    with tile.TileContext(nc) as tc:
        with (
            tc.tile_pool(name="const", bufs=1) as cpool,
            tc.tile_pool(name="state", bufs=1) as spool,
            tc.tile_pool(name="work", bufs=3) as wpool,
            tc.tile_pool(name="outp", bufs=4) as opool,
            tc.tile_pool(name="psumG", bufs=2, space="PSUM") as psG,
            tc.tile_pool(name="psumA", bufs=2, space="PSUM") as psA,
            tc.tile_pool(name="psumW", bufs=2, space="PSUM") as psW,
        ):
            # ---- load constants ----
            def load(dram, shape, dtype, tag):
                t_ = cpool.tile(list(shape), dtype, tag=tag)
                nc.sync.dma_start(out=t_[...], in_=dram[...])
                return t_

            wih = {d: load(wih_d[d], [E, 6, 128], bf16, f"wih{d}") for d in "fb"}
            awih = {d: load(awih_d[d], [E, 2, 128], bf16, f"awih{d}") for d in "fb"}
            wwih = {d: load(wwih_d[d], [E, 6, 128], bf16, f"wwih{d}") for d in "fb"}
            whh = {d: load(whh_d[d], [128, 2, 6, 128], bf16, f"whh{d}") for d in "fb"}
            wwhh = {d: load(wwhh_d[d], [128, 2, 6, 128], bf16, f"wwhh{d}") for d in "fb"}
            awhh = {d: load(awhh_d[d], [128, 2, 2, 128], bf16, f"awhh{d}") for d in "fb"}
            bg = {d: load(bg_d[d], [6, 128], bf16, f"bg{d}") for d in "fb"}
            ba = {d: load(ba_d[d], [2, 128], bf16, f"ba{d}") for d in "fb"}
            bw_ = {d: load(bw_d[d], [6, 128], bf16, f"bw{d}") for d in "fb"}
            sel2 = load(sel2_d, [2, 64], bf16, "sel2")
            selw = load(selw_d, [6, 192], bf16, "selw")
            tagw = load(tagw_d, [128, 2, 2, 32], bf16, "tagw")

            maskm = cpool.tile([128, T, LANES], i8, tag="maskm")
            maskw = cpool.tile([128, T, LANES], i8, tag="maskw")
            for md, mt in ((maskm_d, maskm), (maskw_d, maskw)):
                src = md[...]
                bsrc = bass.AP(tensor=src.tensor, offset=src.offset,
                               ap=[[0, 128]] + list(src.ap))
                nc.sync.dma_start(out=mt[...], in_=bsrc)

            # absorb the mask-DMA completion wait on DVE's vector clock here:
            # copy_predicated (3-AP ISA struct) has only ONE sync-wait slot.
            mwarm = cpool.tile([128, LANES], i8, tag="mwarm")
            nc.vector.tensor_copy(mwarm[...], maskm[:, 0, :])
            nc.vector.tensor_copy(mwarm[...], maskw[:, 0, :])

            x_T = load(xT_d, [128, NIDX], bf16, "xT")
            we_T = load(weT_d, [128, NIDX], bf16, "weT")

            # ---- states (per-parity tiles; index [t % 2]) ----
            h_hist = spool.tile([128, T + 1, 2, 64], bf16)
            c_a = spool.tile([128, 2, 64], f32, tag="c_a")
            c_b = spool.tile([128, 2, 64], f32, tag="c_b")
            q_a = spool.tile([128, 2, 64], f32, tag="q_a")
            q_b = spool.tile([128, 2, 64], f32, tag="q_b")
            pc_a = spool.tile([128, 2, 64], bf16, tag="pc_a")
            pc_b = spool.tile([128, 2, 64], bf16, tag="pc_b")
            c2 = [c_a, c_b]
            q2 = [q_a, q_b]
            pc2 = [pc_a, pc_b]
            nc.vector.memset(h_hist[:, 0, :, :], 0.0)
            for i in range(2):
                nc.vector.memset(c2[i][...], 0.0)
                nc.vector.memset(q2[i][...], 0.0)
                nc.vector.memset(pc2[i][...], 0.0)

            DIRS = (("f", 0), ("b", 32))

            def xcol(tile_, t, l0, n=32):
                return tile_[:, t * LANES + l0: t * LANES + l0 + n]

            def emit_pg_xb(pg, tt):
                """pgm(tt) group open: bias + x-side (no recurrent deps).
                Chunk order i(0,1) o(2,3) g(4,5)."""
                first = True
                for d, l0 in DIRS:
                    nc.tensor.matmul(pg[:, :, l0:l0 + 32], bg[d][...], selw[...],
                                     start=first, stop=False)
                    first = False
                for d, l0 in DIRS:
                    for m in range(6):
                        nc.tensor.matmul(pg[:, m:m + 1, l0:l0 + 32],
                                         wih[d][:, m, :], xcol(x_T, tt, l0),
                                         start=False, stop=False)

            def emit_pg_h(pg, tt):
                """whh into pgm(tt); reads h(tt-1) = slot tt. Emit AFTER the
                h-mul so program order gives RAW, not WAR. Closes the group."""
                n = 0
                for d, l0 in DIRS:
                    for kc in range(2):
                        for m in range(6):
                            n += 1
                            nc.tensor.matmul(pg[:, m:m + 1, l0:l0 + 32],
                                             whh[d][:, kc, m, :],
                                             h_hist[:, tt, kc, l0:l0 + 32],
                                             start=False, stop=(n == 24))

            def emit_pga_xb(pa, tt):
                """pga(tt) group open: ab bias + aWih x-side."""
                first = True
                for d, l0 in DIRS:
                    nc.tensor.matmul(pa[:, :, l0:l0 + 32], ba[d][...], sel2[...],
                                     start=first, stop=False)
                    first = False
                for d, l0 in DIRS:
                    for m in range(2):
                        nc.tensor.matmul(pa[:, m:m + 1, l0:l0 + 32],
                                         awih[d][:, m, :], xcol(x_T, tt, l0),
                                         start=False, stop=False)

            def emit_pga_h(pa, tt):
                """aWhh*pc(tt-2) into pga(tt); pc parity tt%2. Emit AFTER
                the pc blend of V2(tt-2); closes the group."""
                n = 0
                par = tt % 2
                for d, l0 in DIRS:
                    for kc in range(2):
                        for m in range(2):
                            n += 1
                            nc.tensor.matmul(pa[:, m:m + 1, l0:l0 + 32],
                                             awhh[d][:, kc, m, :],
                                             pc2[par][:, kc, l0:l0 + 32],
                                             start=False, stop=(n == 8))

            def emit_pw(pw, t):
                """pw(t) group: bias + word-emb x-side + wwhh (h(t) = slot
                t+1); stop on last wwhh."""
                first = True
                for d, l0 in DIRS:
                    nc.tensor.matmul(pw[:, :, l0:l0 + 32], bw_[d][...], selw[...],
                                     start=first, stop=False)
                    first = False
                for d, l0 in DIRS:
                    for m in range(6):
                        nc.tensor.matmul(pw[:, m:m + 1, l0:l0 + 32],
                                         wwih[d][:, m, :], xcol(we_T, t, l0),
                                         start=False, stop=False)
                n = 0
                for d, l0 in DIRS:
                    for kc in range(2):
                        for m in range(6):
                            n += 1
                            nc.tensor.matmul(pw[:, m:m + 1, l0:l0 + 32],
                                             wwhh[d][:, kc, m, :],
                                             h_hist[:, t + 1, kc, l0:l0 + 32],
                                             start=False, stop=(n == 24))

            # prologue: pg(0)/pga(0) (h(-1)=0 slot 0, pc(-2)=0 parity 0)
            pg = psG.tile([128, 6, 64], f32, tag="pg")
            emit_pg_xb(pg, 0)
            emit_pg_h(pg, 0)
            pa = psA.tile([128, 2, 64], f32, tag="pa")
            emit_pga_xb(pa, 0)
            emit_pga_h(pa, 0)
            sal_cur = wpool.tile([128, 2, 64], f32, tag="sal")
            nc.scalar.activation(sal_cur[...], pa[...], Sig)

            pw_prev = None
            sw_prev = None
            tw_prev = None

            for t in range(T):
                p = t % 2
                m2 = _chunk_bcast(maskm[:, t, :])

                # pc(t-1) staging copy: no deps on this step, run early
                if pw_prev is not None:
                    nc.gpsimd.tensor_copy(pc2[1 - p][...], pc2[p][...])

                # ---- tensor: pg(t+1)/pga(t+1) bias+x (execute early) ----
                if t < T - 1:
                    pg2 = psG.tile([128, 6, 64], f32, tag="pg")
                    emit_pg_xb(pg2, t + 1)
                    pa2 = psA.tile([128, 2, 64], f32, tag="pa")
                    emit_pga_xb(pa2, t + 1)

                # ---- V1(t) scalar front + DVE chain ----
                si = wpool.tile([128, 2, 64], f32)    # sig(i)
                gt = wpool.tile([128, 2, 64], f32)
                nc.scalar.activation(si[...], pg[:, 0:2, :], Sig)
                nc.scalar.activation(gt[...], pg[:, 4:6, :], Tanh)

                # spre = sig(i) - sig(alpha) as fused (al*-1)+i
                spre = wpool.tile([128, 2, 64], f32)
                nc.vector.scalar_tensor_tensor(
                    spre[...], sal_cur[...], -1.0, si[...],
                    mybir.AluOpType.mult, mybir.AluOpType.add)
                s_t = wpool.tile([128, 2, 64], bf16)
                nc.scalar.activation(s_t[...], spre[...], Sig)
                so_t = wpool.tile([128, 2, 64], f32)
                nc.scalar.activation(so_t[...], pg[:, 2:4, :], Sig)

                # V2(t-1) scalar acts slot in after s (pw(t-1) completed by now)
                if pw_prev is not None:
                    sw_prev = wpool.tile([128, 4, 64], f32)
                    tw_prev = wpool.tile([128, 2, 64], f32)
                    nc.scalar.activation(sw_prev[...], pw_prev[:, 0:4, :], Sig)
                    nc.scalar.activation(tw_prev[...], pw_prev[:, 4:6, :], Tanh)

                # e = select(m, s, sig(i))
                e = wpool.tile([128, 2, 64], f32)
                nc.vector.tensor_copy(e[...], si[...])
                nc.vector.copy_predicated(e[...], m2, s_t[...])

                # gq = g - q(t) on gpsimd (off the DVE queue)
                gq = wpool.tile([128, 2, 64], f32)
                nc.gpsimd.tensor_sub(gq[...], gt[...], q2[p][...])
                ed = wpool.tile([128, 2, 64], f32)
                nc.vector.tensor_mul(ed[...], e[...], gq[...])
                nc.vector.tensor_add(c2[p][...], q2[p][...], ed[...])
                if t <= T - 2:
                    nc.gpsimd.tensor_copy(q2[1 - p][...], c2[p][...])
                tc_ = wpool.tile([128, 2, 64], f32)
                nc.scalar.activation(tc_[...], c2[p][...], Tanh)
                nc.vector.tensor_mul(h_hist[:, t + 1, :, :], so_t[...], tc_[...])

                # ---- V2(t-1) on DVE after the h-mul (keeps chain clean) ----
                if pw_prev is not None:
                    t1 = wpool.tile([128, 2, 64], f32)
                    t2 = wpool.tile([128, 2, 64], f32)
                    cw = wpool.tile([128, 2, 64], bf16)
                    nc.vector.tensor_mul(t1[...], sw_prev[:, 2:4, :], c2[1 - p][...])
                    nc.vector.tensor_mul(t2[...], sw_prev[:, 0:2, :], tw_prev[...])
                    nc.vector.tensor_add(cw[...], t1[...], t2[...])
                    w2p = _chunk_bcast(maskw[:, t - 1, :])
                    nc.vector.copy_predicated(pc2[1 - p][...], w2p, cw[...])

                # ---- tensor: close pg(t+1) [whh after h-mul]; close
                # pga(t+1) [awhh after pc blend] + sig(alpha)(t+1);
                # pw(t) behind them in the queue ----
                if t < T - 1:
                    emit_pg_h(pg2, t + 1)
                    emit_pga_h(pa2, t + 1)
                    sal_next = wpool.tile([128, 2, 64], f32, tag="sal")
                    nc.scalar.activation(sal_next[...], pa2[...], Sig)
                if t <= T - 3:
                    pw = psW.tile([128, 6, 64], f32, tag="pw")
                    emit_pw(pw, t)
                else:
                    pw = None

                # ---- q(t+1) = select(m(t+1), pc(t-1), c(t)) -> parity 1-p ----
                if t <= T - 2:
                    m2n = _chunk_bcast(maskm[:, t + 1, :])
                    nc.vector.copy_predicated(q2[1 - p][...], m2n,
                                              pc2[1 - p][...])

                pw_prev = pw
                if t < T - 1:
                    pg = pg2
                    sal_cur = sal_next

            # ---- tag projection: out[d, label, tau*32+lane] ----
            nchunks = (T + NT - 1) // NT
            for di, (d, l0) in enumerate(DIRS):
                for n in range(nchunks):
                    nt_cur = min(NT, T - n * NT)
                    pt = psW.tile([32, NT, 32], f32, tag="ptag")
                    for kc in range(2):
                        nc.tensor.matmul(
                            pt[:, 0:nt_cur, :], tagw[:, di, kc, :],
                            h_hist[:, 1 + n * NT: 1 + n * NT + nt_cur, kc,
                                   l0:l0 + 32],
                            start=(kc == 0), stop=(kc == 1))
                    ob = opool.tile([32, NT, 32], f32)
                    nc.vector.tensor_copy(ob[:, 0:nt_cur, :], pt[:, 0:nt_cur, :])
                    nc.sync.dma_start(
                        out=out_d[di, :, n * NT * 32: (n * NT + nt_cur) * 32],
                        in_=ob[:, 0:nt_cur, :])

    nc.compile()
    return nc


# ------------------------- host side -------------------------

def _window_start(k):
    return OWNK * k


def _masks_for_window(wlen_win):
    """wlen_win [32, T] int -> merge mask m [T,32], has-word hw [T,32] f32,
    replicating the truncated-from-zero pcnt/pvalid recurrence."""
    n = wlen_win.shape[0]
    pcnt = np.full((n,), -1, np.int64)
    pvalid = np.zeros((n,), bool)
    m = np.zeros((T, n), np.float32)
    hw = np.zeros((T, n), np.float32)
    for t in range(T):
        mg = pvalid & (pcnt == 0)
        m[t] = mg.astype(np.float32)
        pvalid = pvalid & ~mg
        pcnt = pcnt - 1
        w = wlen_win[:, t] >= 2
        hw[t] = w.astype(np.float32)
        pcnt = np.where(w, wlen_win[:, t] - 1, pcnt)
        pvalid = pvalid | w
    return m, hw


def _weight_tiles(Wx, Whh, aWx, aWhh, wWx, wWhh, b, ab, wb):
    r = {}
    r["wih"] = np.ascontiguousarray(Wx.reshape(E, 6, 128)).astype(bf)
    r["awih"] = np.ascontiguousarray(aWx.reshape(E, 2, 128)).astype(bf)
    r["wwih"] = np.ascontiguousarray(wWx.reshape(E, 6, 128)).astype(bf)
    r["whh"] = np.ascontiguousarray(
        Whh.reshape(2, 128, 6, 128).transpose(1, 0, 2, 3)).astype(bf)
    r["wwhh"] = np.ascontiguousarray(
        wWhh.reshape(2, 128, 6, 128).transpose(1, 0, 2, 3)).astype(bf)
    r["awhh"] = np.ascontiguousarray(
        aWhh.reshape(2, 128, 2, 128).transpose(1, 0, 2, 3)).astype(bf)
    r["biasg"] = b.reshape(6, 128).astype(bf)
    r["biasa"] = ab.reshape(2, 128).astype(bf)
    r["biasw"] = wb.reshape(6, 128).astype(bf)
    return r


def _prep(inputs):
    inputs = {k: np.asarray(v) for k, v in inputs.items()}
    cids = inputs["component_ids"].astype(np.int64)
    skip = inputs["skip_input"].astype(np.int64)
    wid, wlen = skip[..., 0], skip[..., 1]

    # reference's skip reversal
    tt = np.arange(S)[None, :]
    valid = wlen > 0
    rev_pos = np.where(valid, S - tt - wlen, S)
    skip_rev = np.zeros((B, S + 1, 2), np.int64)
    bidx = np.broadcast_to(np.arange(B)[:, None], (B, S))
    skip_rev[bidx, rev_pos] = skip * valid[..., None]
    skip_rev = skip_rev[:, :S]
    cids_r = cids[:, ::-1]
    wid_r, wlen_r = skip_rev[..., 0], skip_rev[..., 1]

    emb_bf = inputs["emb"].astype(bf)
    emb_bf32 = emb_bf.astype(np.float32)

    wt = {}
    for d, pre in (("f", "fw_"), ("b", "bw_")):
        a = [inputs[pre + n] for n in
             ["Wih", "Whh", "aWih", "aWhh", "wWih", "wWhh", "b", "ab", "wb"]]
        wt[d] = _weight_tiles(*a)

    sel2 = np.zeros((2, 64), np.float32)
    for c in range(2):
        sel2[c, c * 32:(c + 1) * 32] = 1.0
    selw = np.zeros((6, 192), np.float32)
    for c in range(6):
        selw[c, c * 32:(c + 1) * 32] = 1.0

    tag = np.zeros((128, 2, 2, 32), np.float32)
    tw = inputs["tag_W"]          # [512, 32]
    for di in range(2):
        for kc in range(2):
            tag[:, di, kc, :] = tw[256 * di + 128 * kc: 256 * di + 128 * (kc + 1), :]

    shared = {"sel2": sel2.astype(bf), "selw": selw.astype(bf),
              "tagw": tag.astype(bf)}
    for d in "fb":
        for nm in ["wih", "awih", "wwih", "whh", "wwhh", "awhh", "biasg",
                   "biasa", "biasw"]:
            shared[f"{nm}_{d}"] = wt[d][nm]

    in_maps = []
    for k in range(NCORES):
        st = _window_start(k)
        xf = cids[:, st:st + T]          # [32, T]
        xb = cids_r[:, st:st + T]
        wf = wid[:, st:st + T]
        wb2 = wid_r[:, st:st + T]
        xflat = np.concatenate([xf.T, xb.T], axis=1).reshape(-1)   # [T*64]
        wflat = np.concatenate([wf.T, wb2.T], axis=1).reshape(-1)
        mf, hf = _masks_for_window(wlen[:, st:st + T])
        mb, hb = _masks_for_window(wlen_r[:, st:st + T])
        mask_m = np.concatenate([mf, mb], axis=1).astype(np.int8)   # [T, 64]
        mask_w = np.concatenate([hf, hb], axis=1).astype(np.int8)
        im = dict(shared)
        im["x_T"] = np.ascontiguousarray(emb_bf32[xflat, :].T).astype(bf)
        im["we_T"] = np.ascontiguousarray(emb_bf32[wflat, :].T).astype(bf)
        im["mask_m"] = mask_m
        im["mask_w"] = mask_w
        in_maps.append(im)
    return in_maps


def _postprocess(results, inputs):
    tag_b = np.asarray(inputs["tag_b"])
    out = np.zeros((B, S, L), np.float32)
    for k in range(NCORES):
        st = _window_start(k)
        t0 = 0 if k == 0 else WARM
        n_own = T if k == 0 else OWNK
        arr = results[k]["out_tags"]              # [2, 32, T*32]
        fwp = arr[0].reshape(L, T, 32).transpose(2, 1, 0)   # [batch, tau, L]
        bwp = arr[1].reshape(L, T, 32).transpose(2, 1, 0)
        gsl = np.arange(n_own) + st + t0
        out[:, gsl, :] += fwp[:, t0:t0 + n_own, :]
        out[:, S - 1 - gsl, :] += bwp[:, t0:t0 + n_own, :]
    return out + tag_b[None, None, :]


def _ensure_ntff_hook():
    """The image's antenv lacks axon_hooks; shim it so trace=True works."""
    import sys
    import types
    try:
        from antenv.axon_hooks import get_axon_ntff_profile_hook  # noqa: F401
        return
    except ImportError:
        pass
    import antenv
    from trn_agent_boot.trn_boot import _ntff_profile_via_ctypes
    mod = types.ModuleType("antenv.axon_hooks")
    _state = {"h": _ntff_profile_via_ctypes("/opt/axon/libaxon_pjrt.so")}
    mod.set_axon_ntff_profile_hook = lambda h: _state.__setitem__("h", h)
    mod.get_axon_ntff_profile_hook = lambda: _state["h"]
    sys.modules["antenv.axon_hooks"] = mod
    antenv.axon_hooks = mod


def run(inputs, trace=False):
    if trace:
        _ensure_ntff_hook()
    if "nc" not in _CACHE:
        _CACHE["nc"] = _build_bass()
    nc = _CACHE["nc"]
    in_maps = _prep(inputs)
    res = run_bass_kernel_spmd(nc, in_maps, core_ids=list(range(NCORES)),
                               trace=trace)
    out = _postprocess(res.results, {k: np.asarray(v) for k, v in inputs.items()})
    return out, res


def kernel(**inputs):
    out, _ = run(inputs, trace=False)
    return out


# revision 15
# speedup vs baseline: 1.0007x; 1.0007x over previous
"""LatticeLSTM (BiLSTM w/ word cells) Trainium2 kernel.

Sharding: time-sharded across 8 cores with balanced windows. Core k
computes local window [63k, 63k+71) of the 512-step scan for ALL 64
lanes (32 batch fw + 32 batch bw). Core 0 owns all 71 of its steps;
cores 1-7 warm up from zero state for W=8 steps and own the last 63.
(Truncation error ~4e-3 rel; coupled forget gate contracts ~0.5/step.)

Key latency optimization (the scan is dependency-bound, no engine is
>40% busy): the word-cell (V2) chain is taken OFF the per-step critical
path via the pc-lag identity: a merge at step t implies no word started
at t-1 (a word starting at t-1 sets pcnt>=1, killing a merge at t), so
every consumer of pc at step t (alpha pre-gate and the merge select)
may read pc(t-2) instead of pc(t-1). The critical cycle is then only
V1(t) -> whh matmuls -> V1(t+1); the word-cell gates, pc update, and
the merge operand q(t+1)=select(m,pc,c) all compute in engine idle time
with ~1.5 steps of slack. pc/c/q are ping-pong buffers (parity t%2).

Device layout: "layout B" -- gate index on SBUF partitions, lanes on
the free dim; weight-stationary matmuls out[gates,lanes] = W^T @ h.
pg PSUM bank chunk order is [i(2) alpha(2) o(2) g(2)] so ONE Sigmoid
covers i+alpha+o and the s=Sig(sig_i - sig_alpha) chain starts early.
Blends use tensor_copy + copy_predicated (masks are 0/1 f32).

Per-iteration emission (software-pipelined): pg(t+1) prep matmuls
[bias-selector, x-side, whh (waits h(t)), alpha (stop)], V1(t) EW chain,
pw(t) matmuls, V2(t-1) EW + q(t+1) blend. Scalar queue order
[Sig(i,al,o), Tanh(g), Sig(s), Sig(iw,fw)(t-1), Tanh(gw)(t-1), Tanh(c)]
keeps every act within its dependency slack.
"""

import numpy as np
import ml_dtypes

import concourse.bass as bass
import concourse.bacc as bacc
import concourse.tile as tile
from concourse import mybir
from concourse.bass_utils import run_bass_kernel_spmd

B, S, E, H, V, L = 32, 512, 128, 256, 21128, 32
NCORES = 8
WARM = 8
T = 64 + 7 * WARM // 8      # 71 local steps per core
OWNK = T - WARM             # 63 owned steps on cores 1-7
LANES = 64                  # 32 fw + 32 bw
NIDX = T * LANES            # gathered rows per table
NT = 12                     # tag matmul steps per chunk

f32 = mybir.dt.float32
bf16 = mybir.dt.bfloat16
i8 = mybir.dt.int8
Sig = mybir.ActivationFunctionType.Sigmoid
Tanh = mybir.ActivationFunctionType.Tanh

bf = ml_dtypes.bfloat16

_CACHE = {}



def _chunk_bcast(ap2, nchunk=2):
    """[128, 64] AP -> [128, nchunk, 64] with zero-stride chunk dim."""
    return bass.AP(tensor=ap2.tensor, offset=ap2.offset,
                   ap=[ap2.ap[0], [0, nchunk], ap2.ap[1]])


def _build_bass():
    nc = bacc.Bacc(None, target_bir_lowering=False)

    def inp(name, shape, dtype):
        return nc.declare_dram_parameter(name, list(shape), dtype, isOutput=False)

    xT_d = inp("x_T", [128, NIDX], bf16)
    weT_d = inp("we_T", [128, NIDX], bf16)
    wih_d = {d: inp(f"wih_{d}", [E, 6, 128], bf16) for d in "fb"}
    awih_d = {d: inp(f"awih_{d}", [E, 2, 128], bf16) for d in "fb"}
    wwih_d = {d: inp(f"wwih_{d}", [E, 6, 128], bf16) for d in "fb"}
    whh_d = {d: inp(f"whh_{d}", [128, 2, 6, 128], bf16) for d in "fb"}
    wwhh_d = {d: inp(f"wwhh_{d}", [128, 2, 6, 128], bf16) for d in "fb"}
    awhh_d = {d: inp(f"awhh_{d}", [128, 2, 2, 128], bf16) for d in "fb"}
    bg_d = {d: inp(f"biasg_{d}", [6, 128], bf16) for d in "fb"}
    ba_d = {d: inp(f"biasa_{d}", [2, 128], bf16) for d in "fb"}
    bw_d = {d: inp(f"biasw_{d}", [6, 128], bf16) for d in "fb"}
    sel2_d = inp("sel2", [2, 2 * 32], bf16)
    selw_d = inp("selw", [6, 6 * 32], bf16)
    maskm_d = inp("mask_m", [T, LANES], i8)
    maskw_d = inp("mask_w", [T, LANES], i8)
    tagw_d = inp("tagw", [128, 2, 2, 32], bf16)

    out_d = nc.declare_dram_parameter("out_tags", [2, 32, T * 32], f32, isOutput=True)

    with tile.TileContext(nc) as tc:
        with (
            tc.tile_pool(name="const", bufs=1) as cpool,
            tc.tile_pool(name="state", bufs=1) as spool,
            tc.tile_pool(name="work", bufs=3) as wpool,
            tc.tile_pool(name="outp", bufs=4) as opool,
            tc.tile_pool(name="psumG", bufs=2, space="PSUM") as psG,
            tc.tile_pool(name="psumA", bufs=2, space="PSUM") as psA,
            tc.tile_pool(name="psumW", bufs=2, space="PSUM") as psW,
        ):
            # ---- load constants ----
            def load(dram, shape, dtype, tag):
                t_ = cpool.tile(list(shape), dtype, tag=tag)
                nc.sync.dma_start(out=t_[...], in_=dram[...])
                return t_

            wih = {d: load(wih_d[d], [E, 6, 128], bf16, f"wih{d}") for d in "fb"}
            awih = {d: load(awih_d[d], [E, 2, 128], bf16, f"awih{d}") for d in "fb"}
            wwih = {d: load(wwih_d[d], [E, 6, 128], bf16, f"wwih{d}") for d in "fb"}
            whh = {d: load(whh_d[d], [128, 2, 6, 128], bf16, f"whh{d}") for d in "fb"}
            wwhh = {d: load(wwhh_d[d], [128, 2, 6, 128], bf16, f"wwhh{d}") for d in "fb"}
            awhh = {d: load(awhh_d[d], [128, 2, 2, 128], bf16, f"awhh{d}") for d in "fb"}
            bg = {d: load(bg_d[d], [6, 128], bf16, f"bg{d}") for d in "fb"}
            ba = {d: load(ba_d[d], [2, 128], bf16, f"ba{d}") for d in "fb"}
            bw_ = {d: load(bw_d[d], [6, 128], bf16, f"bw{d}") for d in "fb"}
            sel2 = load(sel2_d, [2, 64], bf16, "sel2")
            selw = load(selw_d, [6, 192], bf16, "selw")
            tagw = load(tagw_d, [128, 2, 2, 32], bf16, "tagw")

            maskm = cpool.tile([128, T, LANES], i8, tag="maskm")
            maskw = cpool.tile([128, T, LANES], i8, tag="maskw")
            for md, mt in ((maskm_d, maskm), (maskw_d, maskw)):
                src = md[...]
                bsrc = bass.AP(tensor=src.tensor, offset=src.offset,
                               ap=[[0, 128]] + list(src.ap))
                nc.sync.dma_start(out=mt[...], in_=bsrc)

            # absorb the mask-DMA completion wait on DVE's vector clock here:
            # copy_predicated (3-AP ISA struct) has only ONE sync-wait slot.
            mwarm = cpool.tile([128, LANES], i8, tag="mwarm")
            nc.vector.tensor_copy(mwarm[...], maskm[:, 0, :])
            nc.vector.tensor_copy(mwarm[...], maskw[:, 0, :])

            x_T = load(xT_d, [128, NIDX], bf16, "xT")
            we_T = load(weT_d, [128, NIDX], bf16, "weT")

            # ---- states (per-parity tiles; index [t % 2]) ----
            h_hist = spool.tile([128, T + 1, 2, 64], bf16)
            c_a = spool.tile([128, 2, 64], f32, tag="c_a")
            c_b = spool.tile([128, 2, 64], f32, tag="c_b")
            q_a = spool.tile([128, 2, 64], f32, tag="q_a")
            q_b = spool.tile([128, 2, 64], f32, tag="q_b")
            pc_a = spool.tile([128, 2, 64], bf16, tag="pc_a")
            pc_b = spool.tile([128, 2, 64], bf16, tag="pc_b")
            c2 = [c_a, c_b]
            q2 = [q_a, q_b]
            pc2 = [pc_a, pc_b]
            nc.vector.memset(h_hist[:, 0, :, :], 0.0)
            for i in range(2):
                nc.vector.memset(c2[i][...], 0.0)
                nc.vector.memset(q2[i][...], 0.0)
                nc.vector.memset(pc2[i][...], 0.0)

            DIRS = (("f", 0), ("b", 32))

            def xcol(tile_, t, l0, n=32):
                return tile_[:, t * LANES + l0: t * LANES + l0 + n]

            def emit_pg_xb(pg, tt):
                """pgm(tt) group open: bias + x-side (no recurrent deps).
                Chunk order i(0,1) o(2,3) g(4,5)."""
                first = True
                for d, l0 in DIRS:
                    nc.tensor.matmul(pg[:, :, l0:l0 + 32], bg[d][...], selw[...],
                                     start=first, stop=False)
                    first = False
                for d, l0 in DIRS:
                    for m in range(6):
                        nc.tensor.matmul(pg[:, m:m + 1, l0:l0 + 32],
                                         wih[d][:, m, :], xcol(x_T, tt, l0),
                                         start=False, stop=False)

            def emit_pg_h(pg, tt):
                """whh into pgm(tt); reads h(tt-1) = slot tt. Emit AFTER the
                h-mul so program order gives RAW, not WAR. Closes the group."""
                n = 0
                for d, l0 in DIRS:
                    for kc in range(2):
                        for m in range(6):
                            n += 1
                            nc.tensor.matmul(pg[:, m:m + 1, l0:l0 + 32],
                                             whh[d][:, kc, m, :],
                                             h_hist[:, tt, kc, l0:l0 + 32],
                                             start=False, stop=(n == 24))

            def emit_pga_xb(pa, tt):
                """pga(tt) group open: ab bias + aWih x-side."""
                first = True
                for d, l0 in DIRS:
                    nc.tensor.matmul(pa[:, :, l0:l0 + 32], ba[d][...], sel2[...],
                                     start=first, stop=False)
                    first = False
                for d, l0 in DIRS:
                    for m in range(2):
                        nc.tensor.matmul(pa[:, m:m + 1, l0:l0 + 32],
                                         awih[d][:, m, :], xcol(x_T, tt, l0),
                                         start=False, stop=False)

            def emit_pga_h(pa, tt):
                """aWhh*pc(tt-2) into pga(tt); pc parity tt%2. Emit AFTER
                the pc blend of V2(tt-2); closes the group."""
                n = 0
                par = tt % 2
                for d, l0 in DIRS:
                    for kc in range(2):
                        for m in range(2):
                            n += 1
                            nc.tensor.matmul(pa[:, m:m + 1, l0:l0 + 32],
                                             awhh[d][:, kc, m, :],
                                             pc2[par][:, kc, l0:l0 + 32],
                                             start=False, stop=(n == 8))

            def emit_pw(pw, t):
                """pw(t) group: bias + word-emb x-side + wwhh (h(t) = slot
                t+1); stop on last wwhh."""
                first = True
                for d, l0 in DIRS:
                    nc.tensor.matmul(pw[:, :, l0:l0 + 32], bw_[d][...], selw[...],
                                     start=first, stop=False)
                    first = False
                for d, l0 in DIRS:
                    for m in range(6):
                        nc.tensor.matmul(pw[:, m:m + 1, l0:l0 + 32],
                                         wwih[d][:, m, :], xcol(we_T, t, l0),
                                         start=False, stop=False)
                n = 0
                for d, l0 in DIRS:
                    for kc in range(2):
                        for m in range(6):
                            n += 1
                            nc.tensor.matmul(pw[:, m:m + 1, l0:l0 + 32],
                                             wwhh[d][:, kc, m, :],
                                             h_hist[:, t + 1, kc, l0:l0 + 32],
                                             start=False, stop=(n == 24))

            # prologue: pg(0)/pga(0) (h(-1)=0 slot 0, pc(-2)=0 parity 0)
            pg = psG.tile([128, 6, 64], f32, tag="pg")
            emit_pg_xb(pg, 0)
            emit_pg_h(pg, 0)
            pa = psA.tile([128, 2, 64], f32, tag="pa")
            emit_pga_xb(pa, 0)
            emit_pga_h(pa, 0)
            sal_cur = wpool.tile([128, 2, 64], f32, tag="sal")
            nc.scalar.activation(sal_cur[...], pa[...], Sig)

            pw_prev = None
            sw_prev = None
            tw_prev = None

            for t in range(T):
                p = t % 2
                m2 = _chunk_bcast(maskm[:, t, :])

                # pc(t-1) staging copy: no deps on this step, run early
                if pw_prev is not None:
                    nc.gpsimd.tensor_copy(pc2[1 - p][...], pc2[p][...])

                # ---- tensor: pg(t+1)/pga(t+1) bias+x (execute early) ----
                if t < T - 1:
                    pg2 = psG.tile([128, 6, 64], f32, tag="pg")
                    emit_pg_xb(pg2, t + 1)
                    pa2 = psA.tile([128, 2, 64], f32, tag="pa")
                    emit_pga_xb(pa2, t + 1)

                # ---- V1(t) scalar front + DVE chain ----
                si = wpool.tile([128, 2, 64], f32)    # sig(i)
                gt = wpool.tile([128, 2, 64], f32)
                nc.scalar.activation(si[...], pg[:, 0:2, :], Sig)
                nc.scalar.activation(gt[...], pg[:, 4:6, :], Tanh)

                # spre = sig(i) - sig(alpha) as fused (al*-1)+i
                spre = wpool.tile([128, 2, 64], f32)
                nc.vector.scalar_tensor_tensor(
                    spre[...], sal_cur[...], -1.0, si[...],
                    mybir.AluOpType.mult, mybir.AluOpType.add)
                s_t = wpool.tile([128, 2, 64], bf16)
                nc.scalar.activation(s_t[...], spre[...], Sig)
                so_t = wpool.tile([128, 2, 64], f32)
                nc.scalar.activation(so_t[...], pg[:, 2:4, :], Sig)

                # V2(t-1) scalar acts slot in after s (pw(t-1) completed by now)
                if pw_prev is not None:
                    sw_prev = wpool.tile([128, 4, 64], f32)
                    tw_prev = wpool.tile([128, 2, 64], f32)
                    nc.scalar.activation(sw_prev[...], pw_prev[:, 0:4, :], Sig)
                    nc.scalar.activation(tw_prev[...], pw_prev[:, 4:6, :], Tanh)

                # e = select(m, s, sig(i))
                e = wpool.tile([128, 2, 64], f32)
                nc.vector.tensor_copy(e[...], si[...])
                nc.vector.copy_predicated(e[...], m2, s_t[...])

                # gq = g - q(t) on gpsimd (off the DVE queue)
                gq = wpool.tile([128, 2, 64], f32)
                nc.gpsimd.tensor_sub(gq[...], gt[...], q2[p][...])
                ed = wpool.tile([128, 2, 64], f32)
                nc.vector.tensor_mul(ed[...], e[...], gq[...])
                nc.vector.tensor_add(c2[p][...], q2[p][...], ed[...])
                if t <= T - 2:
                    nc.gpsimd.tensor_copy(q2[1 - p][...], c2[p][...])
                tc_ = wpool.tile([128, 2, 64], f32)
                nc.scalar.activation(tc_[...], c2[p][...], Tanh)

                # ---- V2(t-1) t1/t2/cw in the DVE idle window while tc runs
                if pw_prev is not None:
                    t1 = wpool.tile([128, 2, 64], f32)
                    t2 = wpool.tile([128, 2, 64], f32)
                    cw = wpool.tile([128, 2, 64], bf16)
                    nc.vector.tensor_mul(t1[...], sw_prev[:, 2:4, :], c2[1 - p][...])
                    nc.vector.tensor_mul(t2[...], sw_prev[:, 0:2, :], tw_prev[...])
                    nc.vector.tensor_add(cw[...], t1[...], t2[...])

                nc.vector.tensor_mul(h_hist[:, t + 1, :, :], so_t[...], tc_[...])

                # ---- pc(t-1) blend right after h ----
                if pw_prev is not None:
                    w2p = _chunk_bcast(maskw[:, t - 1, :])
                    nc.vector.copy_predicated(pc2[1 - p][...], w2p, cw[...])

                # ---- tensor: close pg(t+1) [whh after h-mul]; close
                # pga(t+1) [awhh after pc blend] + sig(alpha)(t+1);
                # pw(t) behind them in the queue ----
                if t < T - 1:
                    emit_pg_h(pg2, t + 1)
                    emit_pga_h(pa2, t + 1)
                    sal_next = wpool.tile([128, 2, 64], f32, tag="sal")
                    nc.scalar.activation(sal_next[...], pa2[...], Sig)
                if t <= T - 3:
                    pw = psW.tile([128, 6, 64], f32, tag="pw")
                    emit_pw(pw, t)
                else:
                    pw = None

                # ---- q(t+1) = select(m(t+1), pc(t-1), c(t)) -> parity 1-p ----
                if t <= T - 2:
                    m2n = _chunk_bcast(maskm[:, t + 1, :])
                    nc.vector.copy_predicated(q2[1 - p][...], m2n,
                                              pc2[1 - p][...])

                pw_prev = pw
                if t < T - 1:
                    pg = pg2
                    sal_cur = sal_next

            # ---- tag projection: out[d, label, tau*32+lane] ----
            nchunks = (T + NT - 1) // NT
            for di, (d, l0) in enumerate(DIRS):
                for n in range(nchunks):
                    nt_cur = min(NT, T - n * NT)
                    pt = psW.tile([32, NT, 32], f32, tag="ptag")
                    for kc in range(2):
                        nc.tensor.matmul(
                            pt[:, 0:nt_cur, :], tagw[:, di, kc, :],
                            h_hist[:, 1 + n * NT: 1 + n * NT + nt_cur, kc,
                                   l0:l0 + 32],
                            start=(kc == 0), stop=(kc == 1))
                    ob = opool.tile([32, NT, 32], f32)
                    nc.vector.tensor_copy(ob[:, 0:nt_cur, :], pt[:, 0:nt_cur, :])
                    nc.sync.dma_start(
                        out=out_d[di, :, n * NT * 32: (n * NT + nt_cur) * 32],
                        in_=ob[:, 0:nt_cur, :])

    nc.compile()
    return nc


# ------------------------- host side -------------------------

def _window_start(k):
    return OWNK * k


def _masks_for_window(wlen_win):
    """wlen_win [32, T] int -> merge mask m [T,32], has-word hw [T,32] f32,
    replicating the truncated-from-zero pcnt/pvalid recurrence."""
    n = wlen_win.shape[0]
    pcnt = np.full((n,), -1, np.int64)
    pvalid = np.zeros((n,), bool)
    m = np.zeros((T, n), np.float32)
    hw = np.zeros((T, n), np.float32)
    for t in range(T):
        mg = pvalid & (pcnt == 0)
        m[t] = mg.astype(np.float32)
        pvalid = pvalid & ~mg
        pcnt = pcnt - 1
        w = wlen_win[:, t] >= 2
        hw[t] = w.astype(np.float32)
        pcnt = np.where(w, wlen_win[:, t] - 1, pcnt)
        pvalid = pvalid | w
    return m, hw


def _weight_tiles(Wx, Whh, aWx, aWhh, wWx, wWhh, b, ab, wb):
    r = {}
    r["wih"] = np.ascontiguousarray(Wx.reshape(E, 6, 128)).astype(bf)
    r["awih"] = np.ascontiguousarray(aWx.reshape(E, 2, 128)).astype(bf)
    r["wwih"] = np.ascontiguousarray(wWx.reshape(E, 6, 128)).astype(bf)
    r["whh"] = np.ascontiguousarray(
        Whh.reshape(2, 128, 6, 128).transpose(1, 0, 2, 3)).astype(bf)
    r["wwhh"] = np.ascontiguousarray(
        wWhh.reshape(2, 128, 6, 128).transpose(1, 0, 2, 3)).astype(bf)
    r["awhh"] = np.ascontiguousarray(
        aWhh.reshape(2, 128, 2, 128).transpose(1, 0, 2, 3)).astype(bf)
    r["biasg"] = b.reshape(6, 128).astype(bf)
    r["biasa"] = ab.reshape(2, 128).astype(bf)
    r["biasw"] = wb.reshape(6, 128).astype(bf)
    return r


def _prep(inputs):
    inputs = {k: np.asarray(v) for k, v in inputs.items()}
    cids = inputs["component_ids"].astype(np.int64)
    skip = inputs["skip_input"].astype(np.int64)
    wid, wlen = skip[..., 0], skip[..., 1]

    # reference's skip reversal
    tt = np.arange(S)[None, :]
    valid = wlen > 0
    rev_pos = np.where(valid, S - tt - wlen, S)
    skip_rev = np.zeros((B, S + 1, 2), np.int64)
    bidx = np.broadcast_to(np.arange(B)[:, None], (B, S))
    skip_rev[bidx, rev_pos] = skip * valid[..., None]
    skip_rev = skip_rev[:, :S]
    cids_r = cids[:, ::-1]
    wid_r, wlen_r = skip_rev[..., 0], skip_rev[..., 1]

    emb_bf = inputs["emb"].astype(bf)
    emb_bf32 = emb_bf.astype(np.float32)

    wt = {}
    for d, pre in (("f", "fw_"), ("b", "bw_")):
        a = [inputs[pre + n] for n in
             ["Wih", "Whh", "aWih", "aWhh", "wWih", "wWhh", "b", "ab", "wb"]]
        wt[d] = _weight_tiles(*a)

    sel2 = np.zeros((2, 64), np.float32)
    for c in range(2):
        sel2[c, c * 32:(c + 1) * 32] = 1.0
    selw = np.zeros((6, 192), np.float32)
    for c in range(6):
        selw[c, c * 32:(c + 1) * 32] = 1.0

    tag = np.zeros((128, 2, 2, 32), np.float32)
    tw = inputs["tag_W"]          # [512, 32]
    for di in range(2):
        for kc in range(2):
            tag[:, di, kc, :] = tw[256 * di + 128 * kc: 256 * di + 128 * (kc + 1), :]

    shared = {"sel2": sel2.astype(bf), "selw": selw.astype(bf),
              "tagw": tag.astype(bf)}
    for d in "fb":
        for nm in ["wih", "awih", "wwih", "whh", "wwhh", "awhh", "biasg",
                   "biasa", "biasw"]:
            shared[f"{nm}_{d}"] = wt[d][nm]

    in_maps = []
    for k in range(NCORES):
        st = _window_start(k)
        xf = cids[:, st:st + T]          # [32, T]
        xb = cids_r[:, st:st + T]
        wf = wid[:, st:st + T]
        wb2 = wid_r[:, st:st + T]
        xflat = np.concatenate([xf.T, xb.T], axis=1).reshape(-1)   # [T*64]
        wflat = np.concatenate([wf.T, wb2.T], axis=1).reshape(-1)
        mf, hf = _masks_for_window(wlen[:, st:st + T])
        mb, hb = _masks_for_window(wlen_r[:, st:st + T])
        mask_m = np.concatenate([mf, mb], axis=1).astype(np.int8)   # [T, 64]
        mask_w = np.concatenate([hf, hb], axis=1).astype(np.int8)
        im = dict(shared)
        im["x_T"] = np.ascontiguousarray(emb_bf32[xflat, :].T).astype(bf)
        im["we_T"] = np.ascontiguousarray(emb_bf32[wflat, :].T).astype(bf)
        im["mask_m"] = mask_m
        im["mask_w"] = mask_w
        in_maps.append(im)
    return in_maps


def _postprocess(results, inputs):
    tag_b = np.asarray(inputs["tag_b"])
    out = np.zeros((B, S, L), np.float32)
    for k in range(NCORES):
        st = _window_start(k)
        t0 = 0 if k == 0 else WARM
        n_own = T if k == 0 else OWNK
        arr = results[k]["out_tags"]              # [2, 32, T*32]
        fwp = arr[0].reshape(L, T, 32).transpose(2, 1, 0)   # [batch, tau, L]
        bwp = arr[1].reshape(L, T, 32).transpose(2, 1, 0)
        gsl = np.arange(n_own) + st + t0
        out[:, gsl, :] += fwp[:, t0:t0 + n_own, :]
        out[:, S - 1 - gsl, :] += bwp[:, t0:t0 + n_own, :]
    return out + tag_b[None, None, :]


def _ensure_ntff_hook():
    """The image's antenv lacks axon_hooks; shim it so trace=True works."""
    import sys
    import types
    try:
        from antenv.axon_hooks import get_axon_ntff_profile_hook  # noqa: F401
        return
    except ImportError:
        pass
    import antenv
    from trn_agent_boot.trn_boot import _ntff_profile_via_ctypes
    mod = types.ModuleType("antenv.axon_hooks")
    _state = {"h": _ntff_profile_via_ctypes("/opt/axon/libaxon_pjrt.so")}
    mod.set_axon_ntff_profile_hook = lambda h: _state.__setitem__("h", h)
    mod.get_axon_ntff_profile_hook = lambda: _state["h"]
    sys.modules["antenv.axon_hooks"] = mod
    antenv.axon_hooks = mod


def run(inputs, trace=False):
    if trace:
        _ensure_ntff_hook()
    if "nc" not in _CACHE:
        _CACHE["nc"] = _build_bass()
    nc = _CACHE["nc"]
    in_maps = _prep(inputs)
    res = run_bass_kernel_spmd(nc, in_maps, core_ids=list(range(NCORES)),
                               trace=trace)
    out = _postprocess(res.results, {k: np.asarray(v) for k, v in inputs.items()})
    return out, res


def kernel(**inputs):
    out, _ = run(inputs, trace=False)
    return out
